# revision 64
# baseline (speedup 1.0000x reference)
"""FAVOR+ (Performer) causal linear attention on 8 Trainium2 NeuronCores.

Problem: B=2, L=2048, H=8, D=64, M=128 random features, fp32 in/out.
Sharding: the 16 (b,h) pairs are data-parallel; each of the 8 cores gets 2
pairs and runs the full feature-map + chunked causal scan for them with no
cross-core communication.

Math per (b,h) pair (C=128 position chunks, 16 chunks), exactly matching the
reference semantics including the +EPS terms (which are NOT negligible here:
typical k' values are within an order of magnitude of EPS=1e-6):
  q'_t = exp(qdash_t - qdiag_t - qmax_t) + EPS     (per-position stabilizer)
  k'_s = exp(kdash_s - kdiag_s - gmax) + EPS       (global stabilizer)
  out_t = (sum_{s<=t} q'_t.k'_s * v_s) / (sum_{s<=t} q'_t.k'_s)
  (the reference's ratio=1/sqrt(M) cancels in num/den and is dropped)

On-device numerics are fp16 (1 cyc/row matmuls, fp16-2x DVE modes, half the
DMA bytes) with three exact algebraic rescalings that keep every
intermediate inside fp16 range:
  - exp carries a constant bias: Xe = exp(dash - 7.5). It cancels because
    the stabilizers divide by the max of the SAME biased exponentials.
  - q' is scaled by BETA=64 and k' by ALPHA=1024 (EPS scaled along): global
    per-side constants that cancel in num/den and lift the S = k'.q'
    products (~1e-12 in reference units) out of fp16-underflow territory.
Validated vs the fp32 reference: rel-to-scale err ~1.4e-3 (gate 2e-2).

Engine layout (all PSUM-touching elementwise on DVE/Act only -- the BIR
verifier rejects GPSIMD-PSUM access):
  PE:   16 per-chunk dash matmuls per side (shared cPT, fp16) into 1-bank
        PSUM strips; per-chunk fp16 transposes batched 4-to-a-bank; scan
        matmuls (masked S^T, S~V+2 inter, dual-parity KV accumulation).
  Act:  batched exp per [C,512] strip with AP bias, exp(-diag) columns,
        half the transpose drains, out = num*recip(den) via Copy+scale.
  DVE:  segmented-AP reduces (diag sums, per-strip q rowmax), mask multiply
        (PSUM->SBUF fp16), all KV snapshots, reciprocals, half the
        transpose drains.
  Pool: squares, all scale+EPS, bcol combines, per-strip global-max pieces
        (SBUF only; Pool is barred from PSUM).
  The k/q stabilizer maxes are computed per exp-strip so they pipeline
  behind the remaining dash matmuls instead of serializing after them.
  xT is shipped as [128, L/2] (two d-halves stacked on partitions) to halve
  per-partition DMA bytes; input DMAs are ordered pair-0-first so the
  pair-0 scan overlaps pair-1 prep.
"""

import numpy as np
from contextlib import ExitStack

import concourse.bass as bass
import concourse.mybir as mybir
from concourse import tile, masks
from concourse.bass_utils import run_bass_kernel_spmd

B, L, H, D, M = 2, 2048, 8, 64, 128
C = 128
NCH = L // C              # 16 chunks
E = D + 1                 # 65: value dim + denominator column
NCORES = 8
PPC = (B * H) // NCORES   # 2 (b,h) pairs per core
EPS = 1e-6
DN = 1.0 / (64.0 ** 0.25)       # data_normalizer c
XBIAS = -7.5                    # constant exp bias (cancels via emax/Gexp)
ALPHA = 1024.0                  # k' global rescale (cancels in num/den)
BETA = 64.0                     # q' global rescale (cancels in num/den)
LN_ALPHA = float(np.log(ALPHA))
LN_BETA = float(np.log(BETA))
F32 = mybir.dt.float32
F16 = mybir.dt.float16
AX = mybir.AxisListType
OP = mybir.AluOpType
AF = mybir.ActivationFunctionType

_cache = {}


def _emit_load(ctx, tc, pools, p, xT, xcm, vaugp):
    nc = tc.nc
    (const, big, small, scr, pdash, ptr, pst, pops, kvps_pool, kvsb_pool,
     psmall) = pools
    HL = L // 2
    kT_sb = big.tile([2 * D, HL], F16, tag="kT", name=f"kT_{p}")
    nc.sync.dma_start(kT_sb[:, 0:HL // 2], xT[p, 0][:, 0:HL // 2])
    nc.sync.dma_start(kT_sb[:, HL // 2:HL], xT[p, 0][:, HL // 2:HL])
    kcm_sb = big.tile([C, NCH * D], F16, tag="kcm", name=f"kcm_{p}")
    nc.sync.dma_start(kcm_sb[:], xcm[p, 0])
    qT_sb = big.tile([2 * D, HL], F16, tag="qT", name=f"qT_{p}")
    qcm_sb = big.tile([C, NCH * D], F16, tag="qcm", name=f"qcm_{p}")
    vaug_sb = big.tile([C, NCH * E], F16, tag="vaug", name=f"vaug_{p}")
    return kT_sb, kcm_sb, vaug_sb, qT_sb, qcm_sb


def _emit_load2(ctx, tc, loads, p, xT, xcm, vaugp):
    nc = tc.nc
    kT_sb, kcm_sb, vaug_sb, qT_sb, qcm_sb = loads[p]
    nc.sync.dma_start(qT_sb[:], xT[p, 1])
    nc.sync.dma_start(qcm_sb[:], xcm[p, 1])
    nc.sync.dma_start(vaug_sb[:], vaugp[p])


def _emit_side(ctx, tc, pools, consts, p, side, xT_sb, xcm_sb):
    """Feature map for one side (k or q) of one pair.

    Returns (Xp_all [C, NCH*M] fp16 position-major primed features or None
    for q, XpT_all [M, L] fp16 feature-major primed features)."""
    nc = tc.nc
    identF16, mask_ut, ones_row, cPT, biasx, biasA, biasB = consts
    (const, big, small, scr, pdash, ptr, pst, pops, kvps_pool, kvsb_pool,
     psmall) = pools
    is_k = side == "k"
    cs = lambda c: slice(c * C, (c + 1) * C)

    # dash: per-chunk matmuls into 1-bank PSUM strips (4 chunks each),
    # shared moving cPT; one batched biased exp per strip
    Xe_all = big.tile([C, NCH * M], F16, tag=f"{side}e", name=f"{side}e_{p}")
    if is_k:
        gmp = small.tile([1, NCH // 4], F32, tag="gmp", name=f"gmp_{p}")
    else:
        emax = small.tile([C, NCH], F16, tag="emax", name=f"emax_{p}")
    for g in range(NCH // 4):
        xd_ps = pdash.tile([C, 4 * C], F32, tag="dash",
                           name=f"dash_{p}{side}{g}")
        for i in range(4):
            c = 4 * g + i
            if c < NCH // 2:
                lhsT, rhs = xT_sb[0:D, cs(c)], cPT[0:D, :]
            else:
                lhsT, rhs = xT_sb[D:2 * D, cs(c - NCH // 2)], cPT[D:2 * D, :]
            nc.tensor.matmul(xd_ps[:, i * C:(i + 1) * C],
                             lhsT=lhsT, rhs=rhs,
                             start=True, stop=True)
        strip = Xe_all[:, g * 4 * C:(g + 1) * 4 * C]
        nc.scalar.activation(strip, xd_ps[:], AF.Exp, bias=biasx[:])
        # per-strip stabilizer reduce, pipelined behind the next strip
        if is_k:
            nc.gpsimd.tensor_reduce(gmp[:, g:g + 1], strip,
                                    axis=AX.XYZWC, op=OP.max)
        else:
            nc.vector.tensor_reduce(
                emax[:, 4 * g:4 * (g + 1)],
                strip.rearrange("p (c m) -> p c m", c=4),
                axis=AX.X, op=OP.max)

    # diag column: square chunk-major x (Pool), segmented add-reduce (DVE)
    xsq = scr.tile([C, NCH * D], F16, tag="xsq", name=f"xsq_{p}{side}")
    nc.gpsimd.tensor_mul(xsq[:], xcm_sb[:], xcm_sb[:])
    xdiag = small.tile([C, NCH], F16, tag="xdiag", name=f"xdiag_{p}{side}")
    with nc.allow_low_precision(reason="sum of 64 fp16 squares; validated "
                                "diag abs err ~2e-3 -> 0.2% weight error"):
        nc.vector.tensor_reduce(xdiag[:],
                                xsq[:].rearrange("p (c d) -> p c d", c=NCH),
                                axis=AX.X, op=OP.add)
    # endx = SCALE * exp(-c^2/2 * sum x^2): the 0.0625 folds c^2/2
    endx = small.tile([C, NCH], F32, tag="endx", name=f"endx_{p}{side}")
    nc.scalar.activation(endx[:], xdiag[:], AF.Exp, scale=-0.0625,
                         bias=(biasA[:] if is_k else biasB[:]))

    bcol = small.tile([C, NCH], F32, tag="bcol", name=f"bcol_{p}{side}")
    if is_k:
        # global stabilizer: 1/max over ALL (position, m) of the biased exp
        gm = small.tile([1, 1], F32, tag="gm", name=f"gm_{p}")
        nc.vector.tensor_reduce(gm[:], gmp[:], axis=AX.X, op=OP.max)
        rg = small.tile([1, 1], F32, tag="rg", name=f"rg_{p}")
        nc.vector.reciprocal(rg[:], gm[:])
        nsb = ptr.tile([C, 1], F32, tag="st", name=f"nsb_{p}")
        nc.tensor.matmul(nsb[:], lhsT=ones_row[:], rhs=rg[:], start=True,
                         stop=True)
        rgb = small.tile([C, 1], F32, tag="rgb", name=f"rgb_{p}")
        nc.vector.tensor_copy(rgb[:], nsb[:])
        nc.gpsimd.tensor_scalar_mul(bcol[:], endx[:], rgb[:])
    else:
        # per-position stabilizer: exp(-diag)/rowmax(exp)
        remax = small.tile([C, NCH], F32, tag="remax", name=f"remax_{p}")
        nc.vector.reciprocal(remax[:], emax[:])
        nc.gpsimd.tensor_mul(bcol[:], endx[:], remax[:])

    # x' = exp * bcol + EPS' (Pool), then PE-transpose each chunk to
    # feature-major; 4 transposed chunks batch into one f16 PSUM tile so a
    # single fp16-2x copy drains them (alternating DVE / Act)
    eps_s = ALPHA * EPS if is_k else BETA * EPS
    XpT_all = big.tile([M, L], F16, tag=f"{side}pT", name=f"{side}pT_{p}")
    Xp_all = big.tile([C, NCH * M], F16, tag=f"{side}p", name=f"{side}p_{p}")
    for g in range(NCH // 4):
        tp4 = pdash.tile([M, 4 * C], F16, tag="dash", name=f"tp4_{p}{side}{g}")
        for i in range(4):
            c = 4 * g + i
            nc.gpsimd.tensor_scalar(out=Xp_all[:, cs(c)],
                                    in0=Xe_all[:, cs(c)],
                                    scalar1=bcol[:, c:c + 1], scalar2=eps_s,
                                    op0=OP.mult, op1=OP.add)
            nc.tensor.transpose(tp4[:, i * C:(i + 1) * C], Xp_all[:, cs(c)],
                                identF16[:])
        dst = XpT_all[:, g * 4 * C:(g + 1) * 4 * C]
        if g % 2 == 0:
            nc.vector.tensor_copy(dst, tp4[:])
        else:
            nc.scalar.activation(dst, tp4[:], AF.Copy)
    return (Xp_all if is_k else None), XpT_all


def _scan_begin(ctx, tc, pools, p):
    nc = tc.nc
    (const, big, small, scr, pdash, ptr, pst, pops, kvps_pool, kvsb_pool,
     psmall) = pools
    out_all = big.tile([C, NCH * D], F16, tag="out_all", name=f"out_all_{p}")
    kv_ps = [kvps_pool.tile([M, E], F32, tag="kvps", name=f"kvps_{p}_{i}")
             for i in range(2)]
    return {"out_all": out_all, "kv_ps": kv_ps, "kv_sb": [None, None]}


def _emit_scan_chunk(ctx, tc, pools, consts, p, state, st8, c, vaug_sb, out):
    nc = tc.nc
    identF16, mask_ut, ones_row, cPT, biasx, biasA, biasB = consts
    (const, big, small, scr, pdash, ptr, pst, pops, kvps_pool, kvsb_pool,
     psmall) = pools
    Kp_all, KpT_all, QpT_all = st8
    out_all, kv_ps, kv_sb = state["out_all"], state["kv_ps"], state["kv_sb"]

    cs = lambda c: slice(c * C, (c + 1) * C)
    cs64 = lambda c: slice(c * D, (c + 1) * D)
    cs65 = lambda c: slice(c * E, (c + 1) * E)

    st_ps = ptr.tile([C, C], F32, tag="st", name=f"st_{p}{c}")
    nc.tensor.matmul(st_ps[:], lhsT=KpT_all[:, cs(c)],
                     rhs=QpT_all[:, cs(c)], start=True, stop=True)
    stm = scr.tile([C, C], F16, tag="stm", name=f"stm_{p}{c}")
    nc.vector.tensor_mul(stm[:], st_ps[:], mask_ut[:])

    ops_ps = pops.tile([C, E], F32, tag="ops", name=f"ops_{p}{c}")
    rhs_list = [kv_sb[par] for par in range(2) if kv_sb[par] is not None]
    nc.tensor.matmul(ops_ps[:], lhsT=stm[:], rhs=vaug_sb[:, cs65(c)],
                     start=True, stop=(not rhs_list))
    for n, kvt in enumerate(rhs_list):
        nc.tensor.matmul(ops_ps[:], lhsT=QpT_all[:, cs(c)], rhs=kvt[:],
                         start=False, stop=(n == len(rhs_list) - 1))

    par = c % 2
    nc.tensor.matmul(kv_ps[par][:],
                     lhsT=Kp_all[:, cs(c)], rhs=vaug_sb[:, cs65(c)],
                     start=(c < 2), stop=(c >= NCH - 2),
                     skip_group_check=True)
    if c < NCH - 1:
        kv_sb[par] = kvsb_pool.tile([M, E], F16, tag="kvsb",
                                    name=f"kvsb_{p}_{c}")
        if c % 8 == 7:
            nc.scalar.activation(kv_sb[par][:], kv_ps[par][:], AF.Copy)
        else:
            nc.vector.tensor_copy(kv_sb[par][:], kv_ps[par][:])

    rc = small.tile([C, 1], F32, tag="rc", name=f"rc_{p}{c}")
    nc.vector.reciprocal(rc[:], ops_ps[:, D:E])
    nc.scalar.activation(out_all[:, cs64(c)], ops_ps[:, 0:D], AF.Copy,
                         scale=rc[:])

    if c == NCH - 1:
        HO = NCH * D // 2
        nc.sync.dma_start(out[p][:, 0:HO], out_all[:, 0:HO])
        nc.sync.dma_start(out[p][:, HO:2 * HO], out_all[:, HO:2 * HO])


def _kernel(ctx, tc, out, xT, xcm, vaugp, cPTd):
    nc = tc.nc
    const = ctx.enter_context(tc.tile_pool(name="const", bufs=1))
    big = ctx.enter_context(tc.tile_pool(name="big", bufs=2))
    small = ctx.enter_context(tc.tile_pool(name="small", bufs=8))
    scr = ctx.enter_context(tc.tile_pool(name="scr", bufs=6))
    pdash = ctx.enter_context(tc.tile_pool(name="pdash", bufs=2, space="PSUM"))
    ptr = ctx.enter_context(tc.tile_pool(name="ptr", bufs=2, space="PSUM"))
    pops = ctx.enter_context(tc.tile_pool(name="pops", bufs=2, space="PSUM"))
    kvps_pool = ctx.enter_context(tc.tile_pool(name="kvps", bufs=2,
                                               space="PSUM"))
    kvsb_pool = ctx.enter_context(tc.tile_pool(name="kvsb", bufs=8))
    pst = psmall = ptr
    pools = (const, big, small, scr, pdash, ptr, pst, pops, kvps_pool,
             kvsb_pool, psmall)

    identF16 = const.tile([128, 128], F16)
    masks.make_identity(nc, identF16[:])
    mask_ut = const.tile([128, 128], F16)
    masks.make_upper_triangular(nc, mask_ut[:], val=1.0, diag=True)
    ones_row = const.tile([1, 128], F32)
    nc.any.memset(ones_row[:], 1.0)
    cPT = const.tile([2 * D, M], F16)
    nc.sync.dma_start(cPT[:], cPTd[:])
    biasx = const.tile([128, 1], F32)
    nc.any.memset(biasx[:], XBIAS)
    biasA = const.tile([128, 1], F32)
    nc.any.memset(biasA[:], LN_ALPHA)
    biasB = const.tile([128, 1], F32)
    nc.any.memset(biasB[:], LN_BETA)
    consts = (identF16, mask_ut, ones_row, cPT, biasx, biasA, biasB)

    loads = []
    for p in range(PPC):
        loads.append(_emit_load(ctx, tc, pools, p, xT, xcm, vaugp))
        _emit_load2(ctx, tc, loads, p, xT, xcm, vaugp)
    for p in range(PPC):
        kT_sb, kcm_sb, vaug_sb, qT_sb, qcm_sb = loads[p]
        Kp, KpT = _emit_side(ctx, tc, pools, consts, p, "k", kT_sb, kcm_sb)
        _, QpT = _emit_side(ctx, tc, pools, consts, p, "q", qT_sb, qcm_sb)
        state = _scan_begin(ctx, tc, pools, p)
        for c in range(NCH):
            _emit_scan_chunk(ctx, tc, pools, consts, p, state, (Kp, KpT, QpT),
                             c, vaug_sb, out)


def _split_multiwaits(nc):
    """The installed walrus encodes at most ONE semaphore wait per
    instruction (EventSemaphore excepted, which takes two).  Hoist extra
    wait conditions onto preceding EventSemaphores on the same engine —
    pure wait instructions, no pipeline flush."""
    fix_id = [0]

    def wait_ev(engine, waits):
        fix_id[0] += 1
        return mybir.InstEventSemaphore(
            name=f"I-waitfix-{fix_id[0]}",
            opcode="EventSemaphore",
            engine=engine,
            ins=[], outs=[],
            sync_info=mybir.SyncInfo(on_wait=list(waits), on_update=[]),
        )

    for fn in nc.m.functions:
        for blk in fn.blocks:
            new_insts = []
            for inst in blk.instructions:
                si = inst.sync_info
                waits = list(si.on_wait) if si is not None else []
                is_ev = type(inst).__name__ == "InstEventSemaphore"
                cap = 2 if is_ev else 1
                if len(waits) > cap:
                    extra, keep = waits[:-cap], waits[-cap:]
                    for i in range(0, len(extra), 2):
                        new_insts.append(wait_ev(inst.engine, extra[i:i + 2]))
                    si.on_wait = keep
                new_insts.append(inst)
            blk.instructions[:] = new_insts


def _build():
    if 'nc' in _cache:
        return _cache['nc']
    nc = bass.Bass("TRN2", target_bir_lowering=False, debug=False,
                   num_devices=NCORES)
    # xT[p, 0]=kT, xT[p, 1]=qT: [64, L] fp16; xcm likewise chunk-major
    xT = nc.dram_tensor("xT", [PPC, 2, 2 * D, L // 2], F16,
                        kind="ExternalInput").ap()
    xcm = nc.dram_tensor("xcm", [PPC, 2, C, NCH * D], F16,
                         kind="ExternalInput").ap()
    vaugp = nc.dram_tensor("vaugp", [PPC, C, NCH * E], F16,
                           kind="ExternalInput").ap()
    cPTd = nc.dram_tensor("cPTd", [2 * D, M], F16, kind="ExternalInput").ap()
    out = nc.dram_tensor("out", [PPC, C, NCH * D], F16,
                         kind="ExternalOutput").ap()
    with tile.TileContext(nc) as tc:
        with ExitStack() as ctx:
            _kernel(ctx, tc, out, xT, xcm, vaugp, cPTd)
    _split_multiwaits(nc)
    _cache['nc'] = nc
    return nc


def kernel(query, key, value, projection_matrix, _trace=False):
    """Full inputs in, full output out. Shards (b,h) pairs across 8 cores."""
    query = np.asarray(query, dtype=np.float32)
    key = np.asarray(key, dtype=np.float32)
    value = np.asarray(value, dtype=np.float32)
    projection_matrix = np.asarray(projection_matrix, dtype=np.float32)

    nc = _build()

    # [B,L,H,D] -> [B*H, L, D] pair-major
    def pairs_ld(x):
        return np.ascontiguousarray(x.transpose(0, 2, 1, 3).reshape(B * H, L, D))

    # chunk-major [B*H, 128, NCH*D]: row p holds [chunk][d] for position p
    def chunkmaj(x_ld):
        return np.ascontiguousarray(
            x_ld.reshape(B * H, NCH, C, D).transpose(0, 2, 1, 3)
            .reshape(B * H, C, NCH * D))

    q_ld = pairs_ld(query)
    k_ld = pairs_ld(key)
    v_ld = pairs_ld(value)
    # stacked [B*H, 2(k,q), 128, L/2] fp16: partitions 0-63 d x first
    # L-half, 64-127 d x second L-half (halves per-partition DMA bytes)
    xT = np.stack([k_ld.transpose(0, 2, 1), q_ld.transpose(0, 2, 1)], axis=1)
    xT = xT.reshape(B * H, 2, D, 2, L // 2).transpose(0, 1, 3, 2, 4)
    xT = np.ascontiguousarray(
        xT.reshape(B * H, 2, 2 * D, L // 2).astype(np.float16))
    xcm = np.stack([chunkmaj(k_ld), chunkmaj(q_ld)], axis=1)
    xcm = np.ascontiguousarray(xcm.astype(np.float16))
    # V with a baked ones column: [B*H, 128, NCH*(D+1)] fp16
    v4 = v_ld.reshape(B * H, NCH, C, D).transpose(0, 2, 1, 3)
    vaug = np.concatenate(
        [v4, np.ones((B * H, C, NCH, 1), dtype=np.float32)], axis=3)
    vaug = np.ascontiguousarray(
        vaug.reshape(B * H, C, NCH * E).astype(np.float16))
    cPT1 = (DN * projection_matrix).T.astype(np.float16)
    cPT = np.ascontiguousarray(np.concatenate([cPT1, cPT1], axis=0))

    in_maps = []
    for r in range(NCORES):
        sl = slice(r * PPC, (r + 1) * PPC)
        in_maps.append({
            "xT": xT[sl], "xcm": xcm[sl], "vaugp": vaug[sl],
            "cPTd": cPT.copy(),
        })

    res = run_bass_kernel_spmd(nc, in_maps, list(range(NCORES)), trace=_trace)
    out_cm = np.empty((B * H, C, NCH * D), dtype=np.float32)
    for r in range(NCORES):
        out_cm[r * PPC:(r + 1) * PPC] = np.asarray(
            res.results[r]["out"], dtype=np.float32)
    # chunk-major -> [B*H, L, D] -> [B, L, H, D]
    out_ld = out_cm.reshape(B * H, C, NCH, D).transpose(0, 2, 1, 3).reshape(
        B * H, L, D)
    full = out_ld.reshape(B, H, L, D).transpose(0, 2, 1, 3)
    if _trace:
        return np.ascontiguousarray(full), res
    return np.ascontiguousarray(full)
